# revision 17
# baseline (speedup 1.0000x reference)
"""Trainium2 Bass kernel for the e-prop gradient fit (nn_Eprop_fit).

Reference computes (B=4, T=300, N=200, NIN=100, K=10):
    dw_in [NIN,N], dw_rec [N,N], dw_out [N,K]
via eligibility traces et[b,t,i,j] = post_term[b,t,j]*pre[b,t,i], exponential
filters over t, and contractions with learning signals.

Reformulation (validated to ~4e-7 rel err vs the jax reference):
  For causal filter F_d(x)[t] = sum_{s<=t} d^{t-s} x[s] and any L:
      sum_t L[t]*F_d(x)[t] = sum_s x[s]*R_d(L)[s],   R_d = anti-causal filter.
  With G = R_lam(L), M = post_term*(G + REG*error2/(B*T)), Mf = R_d(M),
  e1f = R_d(error1):
      dw_in  = sum_{b,s} x[b,s,:]^T      Mf[b,s,:]
      dw_rec = sum_{b,s} z_prev[b,s,:]^T Mf[b,s,:]   (diagonal zeroed)
      dw_out = sum_{b,s} z[b,s,:]^T      e1f[b,s,:]
  post_term[t,j] = psi[t,j] * [no spike in z[t-4..t-1, j]],
  psi = 0.3*relu(1-|(v-thr)/thr|)/thr.

Sharding: 8 cores = (batch b in 0..3) x (post-half h in 0..1); host sums
partials over b and concatenates over h (own half packed first so one SPMD
program serves both halves). Time is REVERSED host-side so the anti-causal
filters become forward scans (tensor_tensor_scan along the free axis). The
z_prev shift is applied host-side to z (zsh[u] = z_u[u+1]); e1f is read
shifted on device.

All weight-gradient contractions fuse into ONE PE matmul per time chunk:
  lhsT = [e1fS | MfT] (stationary, 110 cols), rhs = [x | zsh_own | zsh_other]
  (moving, 300 cols), accumulating dwT[110, 300] whose blocks are the
  TRANSPOSED gradients (host transposes on gather), plus a rank-1 correction
  restoring the u=0 term of dw_out dropped by the shift.
"""

import numpy as np

import concourse.bass as bass
import concourse.tile as tile
from concourse import bacc, mybir
from concourse.bass_utils import run_bass_kernel_spmd
from concourse.masks import make_identity

# problem constants (hardcoded per harness contract)
B, T, N, NIN, K = 4, 300, 200, 100, 10
NH = N // 2          # post-half per core
Q = 3                # time chunks
TC = T // Q          # 100
THRESHOLD = 0.6
DECAY = 0.8
N_REF = 5
REG = 300.0
N_CORES = 8
N_WARM = 4           # PE warm-up matmuls during the input-DMA wait

F32 = mybir.dt.float32
Alu = mybir.AluOpType
Act = mybir.ActivationFunctionType

# packed input layout
FEAT_W = 2 * T + N_REF              # vT | zT | zero pad(4) | c2 -> [NH, 605]
LHS_W = NIN + N                     # x | zsh_own | zsh_other -> [TC, Q, 300]
SMALL_W = T + 1 + 2 * NH + K        # e1T | 0 | w_outT | zrow | e1row
RP_W = K + NH                       # [e1fS | MfT] stationary operand
OUT_P, OUT_W = RP_W, LHS_W          # [110, 300] transposed-gradient pack


def _build_program():
    nc = bacc.Bacc("TRN2", target_bir_lowering=False, debug=False,
                   num_devices=N_CORES)

    feat_d = nc.dram_tensor("feat", [NH, FEAT_W], F32, kind="ExternalInput")
    lhs_d = nc.dram_tensor("lhsT", [TC, Q, LHS_W], F32, kind="ExternalInput")
    small_d = nc.dram_tensor("small", [K, SMALL_W], F32, kind="ExternalInput")
    out_d = nc.dram_tensor("out", [OUT_P, OUT_W], F32, kind="ExternalOutput")

    with tile.TileContext(nc) as tc:
        with (
            tc.tile_pool(name="const", bufs=1) as const,
            tc.tile_pool(name="work", bufs=1) as work,
            tc.tile_pool(name="psA", bufs=4, space="PSUM") as psA,
            tc.tile_pool(name="psW", bufs=1, space="PSUM") as psW,
            tc.tile_pool(name="psAcc", bufs=1, space="PSUM") as psAcc,
        ):
            # ---- inputs (all HWDGE): small gates L; lhs chunked so the
            # fused matmuls can start as each chunk lands ----
            small = work.tile([K, SMALL_W], F32, tag="small")
            lhs = work.tile([TC, Q, LHS_W], F32, tag="lhs")
            feat = work.tile([NH, FEAT_W], F32, tag="feat")
            nc.sync.dma_start(out=small[:], in_=small_d.ap())
            nc.scalar.dma_start(out=feat[:, 0:T], in_=feat_d.ap()[:, 0:T])
            nc.sync.dma_start(out=feat[:, T:FEAT_W],
                              in_=feat_d.ap()[:, T:FEAT_W])
            for q in range(Q):
                nc.sync.dma_start(out=lhs[:, q, :], in_=lhs_d.ap()[:, q, :])

            vT = feat[:, 0:T]
            c2 = feat[:, FEAT_W - 1:FEAT_W]
            e1T = small[:, 0:T]
            whT = small[:, T + 1:T + 1 + NH]
            zrow = small[0:1, T + 1 + NH:T + 1 + 2 * NH]
            e1row = small[0:1, T + 1 + 2 * NH:SMALL_W]

            # ---- constants ----
            dk = const.tile([NH, T + 1], F32, tag="dk")
            nc.vector.memset(dk[:], DECAY)
            bm1 = const.tile([NH, 1], F32, tag="bm1")
            nc.vector.memset(bm1[:], -1.0)
            scr = const.tile([NH, 1], F32, tag="scr")
            ident = const.tile([NH, NH], F32, tag="ident")
            make_identity(nc, ident[:])

            # early dummy activation so the ACT table loads during DMA wait
            nc.scalar.activation(scr[:], bm1[:], Act.Abs)

            # PE warm-up during the DMA wait (HAM un-throttle)
            warm_ps = psW.tile([NH, 128], F32, tag="warm")
            for _ in range(N_WARM):
                nc.tensor.matmul(warm_ps[:], dk[:, 0:NH], dk[:, 0:128],
                                 start=True, stop=True)

            # ---- learning signals L[j,u] = sum_k w_out[j,k] e1[u,k] ----
            L_ps = psA.tile([NH, T], F32, tag="psA")
            nc.tensor.matmul(L_ps[:], whT, e1T, start=True, stop=True)

            # e1f scan [K, T+1]; col T is 0.8*e1f[T-1] (small col T is 0) —
            # harmless: the shifted read at u=T-1 meets zsh[T-1]=0.
            e1fp = work.tile([K, T + 1], F32, tag="e1fp")
            nc.vector.tensor_tensor_scan(
                out=e1fp[:], data0=dk[:K, :], data1=small[:, 0:T + 1],
                initial=0.0, op0=Alu.mult, op1=Alu.add)

            # ---- refractory: r[j,u] = sum_{w=1..4} z[j,u+w] (z>=0, padded)
            fz = feat[:, T + 1:T + 1 + T]
            cstride = fz.ap[1][0]
            win = bass.AP(tensor=fz.tensor, offset=fz.offset,
                          ap=[fz.ap[0], [cstride, T], [cstride, N_REF - 1]])
            r = work.tile([NH, T], F32, tag="r")
            nc.vector.tensor_reduce(r[:], win, mybir.AxisListType.X, Alu.add)

            # G = forward scan (in reversed time) of L: g = 0.8*g + L[u]
            G = work.tile([NH, T], F32, tag="G")
            nc.vector.tensor_tensor_scan(
                out=G[:], data0=dk[:, 0:T], data1=L_ps[:], initial=0.0,
                op0=Alu.mult, op1=Alu.add)

            # ---- psi (without the 0.5 factor; host rescales) ----
            psi_a = work.tile([NH, T], F32, tag="psi_a")
            nc.scalar.activation(psi_a[:], vT, Act.Abs,
                                 bias=bm1[:], scale=1.0 / THRESHOLD)
            psi = work.tile([NH, T], F32, tag="psi")
            nc.scalar.activation(psi[:], psi_a[:], Act.Relu,
                                 bias=1.0, scale=-1.0)

            # post = (r == 0) * psi;  M = post * (G + c2)
            post = work.tile([NH, T], F32, tag="post")
            nc.vector.scalar_tensor_tensor(
                out=post[:], in0=r[:], scalar=0.0, in1=psi[:],
                op0=Alu.is_equal, op1=Alu.mult)
            M = work.tile([NH, T], F32, tag="M")
            nc.vector.scalar_tensor_tensor(
                out=M[:], in0=G[:], scalar=c2, in1=post[:],
                op0=Alu.add, op1=Alu.mult)
            # Mf = scan(M), chunked+chained so transposes start early
            Mf = work.tile([NH, T], F32, tag="Mf")
            for q in range(Q):
                nc.vector.tensor_tensor_scan(
                    out=Mf[:, q * TC:(q + 1) * TC], data0=dk[:, 0:TC],
                    data1=M[:, q * TC:(q + 1) * TC],
                    initial=0.0 if q == 0 else Mf[:, q * TC - 1:q * TC],
                    op0=Alu.mult, op1=Alu.add)

            # ---- stationary operand rhsP = [e1fS | MfT] per chunk ----
            rhsP = work.tile([TC, Q, RP_W], F32, tag="rhsP")
            dwT_ps = psAcc.tile([RP_W, LHS_W], F32, tag="dwT")
            for q in range(Q):
                trE = psA.tile([TC, K], F32, tag="psA")
                nc.tensor.transpose(
                    trE[:], e1fp[:, q * TC + 1:(q + 1) * TC + 1],
                    ident[:K, :K])
                nc.scalar.copy(rhsP[:, q, 0:K], trE[:])
            for q in range(Q):
                trT = psA.tile([TC, NH], F32, tag="psA")
                nc.tensor.transpose(trT[:], Mf[:, q * TC:(q + 1) * TC],
                                    ident[:])
                nc.scalar.copy(rhsP[:, q, K:RP_W], trT[:])
                nc.tensor.matmul(dwT_ps[:], rhsP[:, q, :], lhs[:, q, :],
                                 start=(q == 0), stop=False)
            # rank-1 correction: dw_outT += e1f[0] (x) z_u[0]
            nc.tensor.matmul(dwT_ps[0:K, NIN:NIN + NH], e1row, zrow,
                             start=False, stop=True)

            # ---- pack transposed gradients and store ----
            # (dw_outT sits inside rows 0:K, cols NIN:NIN+NH of dwT)
            outt = work.tile([OUT_P, OUT_W], F32, tag="outt")
            nc.vector.tensor_copy(outt[0:64, :], dwT_ps[0:64, :])
            nc.scalar.copy(outt[64:OUT_P, :], dwT_ps[64:RP_W, :])
            nc.sync.dma_start(out=out_d.ap(), in_=outt[:])

    nc.compile()
    return nc


_NC_CACHE = None


def _get_nc():
    global _NC_CACHE
    if _NC_CACHE is None:
        _NC_CACHE = _build_program()
    return _NC_CACHE


def _prep_core_inputs(v, z, x, error1, error2, w_out, b, h):
    jsl = slice(h * NH, (h + 1) * NH)
    osl = slice((1 - h) * NH, (2 - h) * NH)
    rev = slice(None, None, -1)

    z_r = z[b, rev, :]                         # [T, N]
    x_r = x[b, rev, :]                         # [T, NIN]
    zsh = np.vstack([z_r[1:], np.zeros((1, N), np.float32)])

    feat = np.zeros((NH, FEAT_W), np.float32)
    feat[:, 0:T] = v[b, rev, jsl].T
    feat[:, T:2 * T] = z_r[:, jsl].T           # cols 600:604 stay zero
    feat[:, FEAT_W - 1] = (REG / (B * T)) * error2[jsl]

    lhs = np.empty((TC, Q, LHS_W), np.float32)
    for q in range(Q):
        rows = slice(q * TC, (q + 1) * TC)
        lhs[:, q, 0:NIN] = x_r[rows]
        lhs[:, q, NIN:NIN + NH] = zsh[rows][:, jsl]
        lhs[:, q, NIN + NH:LHS_W] = zsh[rows][:, osl]

    small = np.zeros((K, SMALL_W), np.float32)
    small[:, 0:T] = error1[b, rev, :].T        # col T stays zero (scan pad)
    small[:, T + 1:T + 1 + NH] = w_out[jsl, :].T
    small[0, T + 1 + NH:T + 1 + 2 * NH] = z_r[0, jsl]       # z_u[0]
    small[0, T + 1 + 2 * NH:SMALL_W] = error1[b, T - 1, :]  # e1f[0]

    return {"feat": feat, "lhsT": np.ascontiguousarray(lhs), "small": small}


def kernel(v, z, x, error1, error2, w_out, _trace=False):
    v = np.asarray(v, np.float32)
    z = np.asarray(z, np.float32)
    x = np.asarray(x, np.float32)
    error1 = np.asarray(error1, np.float32)
    error2 = np.asarray(error2, np.float32)
    w_out = np.asarray(w_out, np.float32)

    nc = _get_nc()
    in_maps = [_prep_core_inputs(v, z, x, error1, error2, w_out, c // 2, c % 2)
               for c in range(N_CORES)]
    res = run_bass_kernel_spmd(nc, in_maps, core_ids=list(range(N_CORES)),
                               trace=_trace)

    dw_in = np.zeros((NIN, N), np.float32)
    dw_rec = np.zeros((N, N), np.float32)
    dw_out = np.zeros((N, K), np.float32)
    for h in range(2):
        jsl = slice(h * NH, (h + 1) * NH)
        osl = slice((1 - h) * NH, (2 - h) * NH)
        s = np.zeros((OUT_P, OUT_W), np.float64)
        for b in range(B):
            s += res.results[2 * b + h]["out"]
        dw_in[:, jsl] = 0.5 * s[K:RP_W, 0:NIN].T
        dw_rec[jsl, jsl] = 0.5 * s[K:RP_W, NIN:NIN + NH].T    # own-half rows
        dw_rec[osl, jsl] = 0.5 * s[K:RP_W, NIN + NH:LHS_W].T  # other-half
        dw_out[jsl, :] = s[0:K, NIN:NIN + NH].T
    np.fill_diagonal(dw_rec, 0.0)

    if _trace:
        return (dw_in, dw_rec, dw_out), res
    return dw_in, dw_rec, dw_out


# revision 18
# speedup vs baseline: 1.0181x; 1.0181x over previous
"""Trainium2 Bass kernel for the e-prop gradient fit (nn_Eprop_fit).

Reference computes (B=4, T=300, N=200, NIN=100, K=10):
    dw_in [NIN,N], dw_rec [N,N], dw_out [N,K]
via eligibility traces et[b,t,i,j] = post_term[b,t,j]*pre[b,t,i], exponential
filters over t, and contractions with learning signals.

Reformulation (validated to ~4e-7 rel err vs the jax reference):
  For causal filter F_d(x)[t] = sum_{s<=t} d^{t-s} x[s] and any L:
      sum_t L[t]*F_d(x)[t] = sum_s x[s]*R_d(L)[s],   R_d = anti-causal filter.
  With G = R_lam(L), M = post_term*(G + REG*error2/(B*T)), Mf = R_d(M),
  e1f = R_d(error1):
      dw_in  = sum_{b,s} x[b,s,:]^T      Mf[b,s,:]
      dw_rec = sum_{b,s} z_prev[b,s,:]^T Mf[b,s,:]   (diagonal zeroed)
      dw_out = sum_{b,s} z[b,s,:]^T      e1f[b,s,:]
  post_term[t,j] = psi[t,j] * [no spike in z[t-4..t-1, j]],
  psi = 0.3*relu(1-|(v-thr)/thr|)/thr.

Sharding: 8 cores = (batch b in 0..3) x (post-half h in 0..1); host sums
partials over b and concatenates over h (own half packed first so one SPMD
program serves both halves). Time is REVERSED host-side so the anti-causal
filters become forward scans (tensor_tensor_scan along the free axis). The
z_prev shift is applied host-side to z (zsh[u] = z_u[u+1]); e1f is read
shifted on device.

All weight-gradient contractions fuse into ONE PE matmul per time chunk:
  lhsT = [e1fS | MfT] (stationary, 110 cols), rhs = [x | zsh_own | zsh_other]
  (moving, 300 cols), accumulating dwT[110, 300] whose blocks are the
  TRANSPOSED gradients (host transposes on gather), plus a rank-1 correction
  restoring the u=0 term of dw_out dropped by the shift.
"""

import numpy as np

import concourse.bass as bass
import concourse.tile as tile
from concourse import bacc, mybir
from concourse.bass_utils import run_bass_kernel_spmd
from concourse.masks import make_identity

# problem constants (hardcoded per harness contract)
B, T, N, NIN, K = 4, 300, 200, 100, 10
NH = N // 2          # post-half per core
Q = 3                # time chunks
TC = T // Q          # 100
THRESHOLD = 0.6
DECAY = 0.8
N_REF = 5
REG = 300.0
N_CORES = 8
N_WARM = 4           # PE warm-up matmuls during the input-DMA wait

F32 = mybir.dt.float32
Alu = mybir.AluOpType
Act = mybir.ActivationFunctionType

# packed input layout
FEAT_W = 2 * T + N_REF              # vT | zT | zero pad(4) | c2 -> [NH, 605]
LHS_W = NIN + N                     # x | zsh_own | zsh_other -> [TC, Q, 300]
SMALL_W = T + 1 + NH                # e1T | 0 | w_outT
RP_W = K + NH                       # [e1fS | MfT] stationary operand
OUT_P, OUT_W = RP_W, LHS_W          # [110, 300] transposed-gradient pack


def _build_program():
    nc = bacc.Bacc("TRN2", target_bir_lowering=False, debug=False,
                   num_devices=N_CORES)

    feat_d = nc.dram_tensor("feat", [NH, FEAT_W], F32, kind="ExternalInput")
    lhs_d = nc.dram_tensor("lhsT", [TC, Q, LHS_W], F32, kind="ExternalInput")
    small_d = nc.dram_tensor("small", [K, SMALL_W], F32, kind="ExternalInput")
    out_d = nc.dram_tensor("out", [OUT_P, OUT_W], F32, kind="ExternalOutput")

    with tile.TileContext(nc) as tc:
        with (
            tc.tile_pool(name="const", bufs=1) as const,
            tc.tile_pool(name="work", bufs=1) as work,
            tc.tile_pool(name="psA", bufs=4, space="PSUM") as psA,
            tc.tile_pool(name="psW", bufs=1, space="PSUM") as psW,
            tc.tile_pool(name="psAcc", bufs=1, space="PSUM") as psAcc,
        ):
            # ---- inputs (all HWDGE): small gates L; lhs chunked so the
            # fused matmuls can start as each chunk lands ----
            small = work.tile([K, SMALL_W], F32, tag="small")
            lhs = work.tile([TC, Q, LHS_W], F32, tag="lhs")
            feat = work.tile([NH, FEAT_W], F32, tag="feat")
            nc.sync.dma_start(out=small[:], in_=small_d.ap())
            nc.sync.dma_start(out=feat[:, T:FEAT_W],
                              in_=feat_d.ap()[:, T:FEAT_W])
            nc.scalar.dma_start(out=feat[:, 0:T], in_=feat_d.ap()[:, 0:T])
            for q in range(Q):
                nc.sync.dma_start(out=lhs[:, q, :], in_=lhs_d.ap()[:, q, :])

            vT = feat[:, 0:T]
            c2 = feat[:, FEAT_W - 1:FEAT_W]
            e1T = small[:, 0:T]
            whT = small[:, T + 1:T + 1 + NH]

            # ---- constants ----
            dk = const.tile([NH, T + 1], F32, tag="dk")
            nc.vector.memset(dk[:], DECAY)
            bm1 = const.tile([NH, 1], F32, tag="bm1")
            nc.vector.memset(bm1[:], -1.0)
            scr = const.tile([NH, 1], F32, tag="scr")
            ident = const.tile([NH, NH], F32, tag="ident")
            make_identity(nc, ident[:])

            # early dummy activation so the ACT table loads during DMA wait
            nc.scalar.activation(scr[:], bm1[:], Act.Abs)

            # PE warm-up during the DMA wait (HAM un-throttle)
            warm_ps = psW.tile([NH, 128], F32, tag="warm")
            for _ in range(N_WARM):
                nc.tensor.matmul(warm_ps[:], dk[:, 0:NH], dk[:, 0:128],
                                 start=True, stop=True)

            # ---- learning signals L[j,u] = sum_k w_out[j,k] e1[u,k] ----
            L_ps = psA.tile([NH, T], F32, tag="psA")
            nc.tensor.matmul(L_ps[:], whT, e1T, start=True, stop=True)

            for _ in range(6):
                nc.tensor.matmul(warm_ps[:], dk[:, 0:NH], dk[:, 0:128],
                                 start=True, stop=True)

            # e1f scan [K, T+1]; col T is 0.8*e1f[T-1] (small col T is 0) —
            # harmless: the shifted read at u=T-1 meets zsh[T-1]=0.
            e1fp = work.tile([K, T + 1], F32, tag="e1fp")
            nc.vector.tensor_tensor_scan(
                out=e1fp[:], data0=dk[:K, :], data1=small[:, 0:T + 1],
                initial=0.0, op0=Alu.mult, op1=Alu.add)

            # ---- refractory: r[j,u] = sum_{w=1..4} z[j,u+w] (z>=0, padded)
            fz = feat[:, T + 1:T + 1 + T]
            cstride = fz.ap[1][0]
            win = bass.AP(tensor=fz.tensor, offset=fz.offset,
                          ap=[fz.ap[0], [cstride, T], [cstride, N_REF - 1]])
            r = work.tile([NH, T], F32, tag="r")
            nc.vector.tensor_reduce(r[:], win, mybir.AxisListType.X, Alu.add)

            # G = forward scan (in reversed time) of L: g = 0.8*g + L[u]
            G = work.tile([NH, T], F32, tag="G")
            nc.vector.tensor_tensor_scan(
                out=G[:], data0=dk[:, 0:T], data1=L_ps[:], initial=0.0,
                op0=Alu.mult, op1=Alu.add)

            # ---- psi (without the 0.5 factor; host rescales) ----
            psi_a = work.tile([NH, T], F32, tag="psi_a")
            nc.scalar.activation(psi_a[:], vT, Act.Abs,
                                 bias=bm1[:], scale=1.0 / THRESHOLD)
            psi = work.tile([NH, T], F32, tag="psi")
            nc.scalar.activation(psi[:], psi_a[:], Act.Relu,
                                 bias=1.0, scale=-1.0)

            # post = (r == 0) * psi;  M = post * (G + c2)
            post = work.tile([NH, T], F32, tag="post")
            nc.vector.scalar_tensor_tensor(
                out=post[:], in0=r[:], scalar=0.0, in1=psi[:],
                op0=Alu.is_equal, op1=Alu.mult)
            M = work.tile([NH, T], F32, tag="M")
            nc.vector.scalar_tensor_tensor(
                out=M[:], in0=G[:], scalar=c2, in1=post[:],
                op0=Alu.add, op1=Alu.mult)
            # Mf = scan(M), chunked+chained so transposes start early
            Mf = work.tile([NH, T], F32, tag="Mf")
            for q in range(Q):
                nc.vector.tensor_tensor_scan(
                    out=Mf[:, q * TC:(q + 1) * TC], data0=dk[:, 0:TC],
                    data1=M[:, q * TC:(q + 1) * TC],
                    initial=0.0 if q == 0 else Mf[:, q * TC - 1:q * TC],
                    op0=Alu.mult, op1=Alu.add)

            # ---- stationary operand rhsP = [e1fS | MfT] per chunk ----
            rhsP = work.tile([TC, Q, RP_W], F32, tag="rhsP")
            dwT_ps = psAcc.tile([RP_W, LHS_W], F32, tag="dwT")
            for q in range(Q):
                trE = psA.tile([TC, K], F32, tag="psA")
                nc.tensor.transpose(
                    trE[:], e1fp[:, q * TC + 1:(q + 1) * TC + 1],
                    ident[:K, :K])
                nc.scalar.copy(rhsP[:, q, 0:K], trE[:])
            for q in range(Q):
                trT = psA.tile([TC, NH], F32, tag="psA")
                nc.tensor.transpose(trT[:], Mf[:, q * TC:(q + 1) * TC],
                                    ident[:])
                nc.scalar.copy(rhsP[:, q, K:RP_W], trT[:])
            for q in range(Q):
                nc.tensor.matmul(dwT_ps[:], rhsP[:, q, :], lhs[:, q, :],
                                 start=(q == 0), stop=(q == Q - 1))

            # ---- pack transposed gradients and store ----
            # (dw_outT sits inside rows 0:K, cols NIN:NIN+NH of dwT)
            outt = work.tile([OUT_P, OUT_W], F32, tag="outt")
            nc.vector.tensor_copy(outt[0:64, :], dwT_ps[0:64, :])
            nc.scalar.copy(outt[64:OUT_P, :], dwT_ps[64:RP_W, :])
            nc.sync.dma_start(out=out_d.ap(), in_=outt[:])

    nc.compile()
    return nc


_NC_CACHE = None


def _get_nc():
    global _NC_CACHE
    if _NC_CACHE is None:
        _NC_CACHE = _build_program()
    return _NC_CACHE


def _prep_core_inputs(v, z, x, error1, error2, w_out, b, h):
    jsl = slice(h * NH, (h + 1) * NH)
    osl = slice((1 - h) * NH, (2 - h) * NH)
    rev = slice(None, None, -1)

    z_r = z[b, rev, :]                         # [T, N]
    x_r = x[b, rev, :]                         # [T, NIN]
    zsh = np.vstack([z_r[1:], np.zeros((1, N), np.float32)])

    feat = np.zeros((NH, FEAT_W), np.float32)
    feat[:, 0:T] = v[b, rev, jsl].T
    feat[:, T:2 * T] = z_r[:, jsl].T           # cols 600:604 stay zero
    feat[:, FEAT_W - 1] = (REG / (B * T)) * error2[jsl]

    lhs = np.empty((TC, Q, LHS_W), np.float32)
    for q in range(Q):
        rows = slice(q * TC, (q + 1) * TC)
        lhs[:, q, 0:NIN] = x_r[rows]
        lhs[:, q, NIN:NIN + NH] = zsh[rows][:, jsl]
        lhs[:, q, NIN + NH:LHS_W] = zsh[rows][:, osl]

    small = np.zeros((K, SMALL_W), np.float32)
    small[:, 0:T] = error1[b, rev, :].T        # col T stays zero (scan pad)
    small[:, T + 1:T + 1 + NH] = w_out[jsl, :].T

    # rank-1 dw_out correction (pure-input term dropped by the zsh shift):
    # dw_out[jsl] += z_u[0] (x) e1f[0] = z[b,T-1,jsl] (x) error1[b,T-1,:]
    corr = np.outer(z_r[0, jsl], error1[b, T - 1, :]).astype(np.float32)

    return {"feat": feat, "lhsT": np.ascontiguousarray(lhs),
            "small": small}, corr


def kernel(v, z, x, error1, error2, w_out, _trace=False):
    v = np.asarray(v, np.float32)
    z = np.asarray(z, np.float32)
    x = np.asarray(x, np.float32)
    error1 = np.asarray(error1, np.float32)
    error2 = np.asarray(error2, np.float32)
    w_out = np.asarray(w_out, np.float32)

    nc = _get_nc()
    prepped = [_prep_core_inputs(v, z, x, error1, error2, w_out, c // 2, c % 2)
               for c in range(N_CORES)]
    in_maps = [p[0] for p in prepped]
    corrs = [p[1] for p in prepped]
    res = run_bass_kernel_spmd(nc, in_maps, core_ids=list(range(N_CORES)),
                               trace=_trace)

    dw_in = np.zeros((NIN, N), np.float32)
    dw_rec = np.zeros((N, N), np.float32)
    dw_out = np.zeros((N, K), np.float32)
    for h in range(2):
        jsl = slice(h * NH, (h + 1) * NH)
        osl = slice((1 - h) * NH, (2 - h) * NH)
        s = np.zeros((OUT_P, OUT_W), np.float64)
        for b in range(B):
            s += res.results[2 * b + h]["out"]
        dw_in[:, jsl] = 0.5 * s[K:RP_W, 0:NIN].T
        dw_rec[jsl, jsl] = 0.5 * s[K:RP_W, NIN:NIN + NH].T    # own-half rows
        dw_rec[osl, jsl] = 0.5 * s[K:RP_W, NIN + NH:LHS_W].T  # other-half
        dw_out[jsl, :] = s[0:K, NIN:NIN + NH].T
        for b in range(B):
            dw_out[jsl, :] += corrs[2 * b + h]
    np.fill_diagonal(dw_rec, 0.0)

    if _trace:
        return (dw_in, dw_rec, dw_out), res
    return dw_in, dw_rec, dw_out


# revision 20
# speedup vs baseline: 1.0502x; 1.0316x over previous
"""Trainium2 Bass kernel for the e-prop gradient fit (nn_Eprop_fit).

Reference computes (B=4, T=300, N=200, NIN=100, K=10):
    dw_in [NIN,N], dw_rec [N,N], dw_out [N,K]
via eligibility traces et[b,t,i,j] = post_term[b,t,j]*pre[b,t,i], exponential
filters over t, and contractions with learning signals.

Reformulation (validated to ~4e-7 rel err vs the jax reference):
  For causal filter F_d(x)[t] = sum_{s<=t} d^{t-s} x[s] and any L:
      sum_t L[t]*F_d(x)[t] = sum_s x[s]*R_d(L)[s],   R_d = anti-causal filter.
  With G = R_lam(L), M = post_term*(G + REG*error2/(B*T)), Mf = R_d(M),
  e1f = R_d(error1):
      dw_in  = sum_{b,s} x[b,s,:]^T      Mf[b,s,:]
      dw_rec = sum_{b,s} z_prev[b,s,:]^T Mf[b,s,:]   (diagonal zeroed)
      dw_out = sum_{b,s} z[b,s,:]^T      e1f[b,s,:]
  post_term[t,j] = psi[t,j] * [no spike in z[t-4..t-1, j]],
  psi = 0.3*relu(1-|(v-thr)/thr|)/thr.

Sharding: 8 cores = (batch b in 0..3) x (post-half h in 0..1); host sums
partials over b and concatenates over h (own half packed first so one SPMD
program serves both halves). Time is REVERSED host-side so the anti-causal
filters become forward scans (tensor_tensor_scan along the free axis). The
z_prev shift is applied host-side to z (zsh[u] = z_u[u+1]); e1f is read
shifted on device.

All weight-gradient contractions fuse into ONE PE matmul per time chunk:
  lhsT = [e1fS | MfT] (stationary, 110 cols), rhs = [x | zsh_own | zsh_other]
  (moving, 300 cols), accumulating dwT[110, 300] whose blocks are the
  TRANSPOSED gradients (host transposes on gather), plus a rank-1 correction
  restoring the u=0 term of dw_out dropped by the shift.
"""

import numpy as np

import concourse.bass as bass
import concourse.tile as tile
from concourse import bacc, mybir
from concourse.bass_utils import run_bass_kernel_spmd
from concourse.masks import make_identity

# problem constants (hardcoded per harness contract)
B, T, N, NIN, K = 4, 300, 200, 100, 10
NH = N // 2          # post-half per core
Q = 3                # time chunks
TC = T // Q          # 100
THRESHOLD = 0.6
DECAY = 0.8
N_REF = 5
REG = 300.0
N_CORES = 8

F32 = mybir.dt.float32
Alu = mybir.AluOpType
Act = mybir.ActivationFunctionType

# packed input layout
FEAT_W = 2 * T + N_REF              # vT | zT | zero pad(4) | c2 -> [NH, 605]
LHS_W = NIN + N                     # x | zsh_own | zsh_other -> [TC, Q, 300]
SMALL_W = T + 1 + NH                # e1T | 0 | w_outT
RP_W = K + NH                       # [e1fS | MfT] stationary operand
OUT_P, OUT_W = RP_W, LHS_W          # [110, 300] transposed-gradient pack


def _build_program():
    nc = bacc.Bacc("TRN2", target_bir_lowering=False, debug=False,
                   num_devices=N_CORES)

    feat_d = nc.dram_tensor("feat", [NH, FEAT_W], F32, kind="ExternalInput")
    lhs_d = nc.dram_tensor("lhsT", [TC, Q, LHS_W], F32, kind="ExternalInput")
    small_d = nc.dram_tensor("small", [K, SMALL_W], F32, kind="ExternalInput")
    out_d = nc.dram_tensor("out", [OUT_P, OUT_W], F32, kind="ExternalOutput")

    with tile.TileContext(nc) as tc:
        with (
            tc.tile_pool(name="const", bufs=1) as const,
            tc.tile_pool(name="work", bufs=1) as work,
            tc.tile_pool(name="psA", bufs=4, space="PSUM") as psA,
            tc.tile_pool(name="psAcc", bufs=1, space="PSUM") as psAcc,
        ):
            # ---- inputs (all HWDGE): small gates L; lhs chunked so the
            # fused matmuls can start as each chunk lands ----
            small = work.tile([K, SMALL_W], F32, tag="small")
            lhs = work.tile([TC, Q, LHS_W], F32, tag="lhs")
            feat = work.tile([NH, FEAT_W], F32, tag="feat")
            nc.sync.dma_start(out=small[:], in_=small_d.ap())
            nc.sync.dma_start(out=feat[:, T:FEAT_W],
                              in_=feat_d.ap()[:, T:FEAT_W])
            nc.scalar.dma_start(out=feat[:, 0:T], in_=feat_d.ap()[:, 0:T])
            for q in range(Q):
                nc.sync.dma_start(out=lhs[:, q, :], in_=lhs_d.ap()[:, q, :])

            vT = feat[:, 0:T]
            c2 = feat[:, FEAT_W - 1:FEAT_W]
            e1T = small[:, 0:T]
            whT = small[:, T + 1:T + 1 + NH]

            # ---- constants ----
            dk = const.tile([NH, T + 1], F32, tag="dk")
            nc.vector.memset(dk[:], DECAY)
            bm1 = const.tile([NH, 1], F32, tag="bm1")
            nc.vector.memset(bm1[:], -1.0)
            scr = const.tile([NH, 1], F32, tag="scr")
            ident = const.tile([NH, NH], F32, tag="ident")
            make_identity(nc, ident[:])

            # early dummy activation so the ACT table loads during DMA wait
            nc.scalar.activation(scr[:], bm1[:], Act.Abs)

            # ---- learning signals L[j,u] = sum_k w_out[j,k] e1[u,k] ----
            L_ps = psA.tile([NH, T], F32, tag="psA")
            nc.tensor.matmul(L_ps[:], whT, e1T, start=True, stop=True)

            # e1f scan [K, T+1]; col T is 0.8*e1f[T-1] (small col T is 0) —
            # harmless: the shifted read at u=T-1 meets zsh[T-1]=0.
            e1fp = work.tile([K, T + 1], F32, tag="e1fp")
            nc.vector.tensor_tensor_scan(
                out=e1fp[:], data0=dk[:K, :], data1=small[:, 0:T + 1],
                initial=0.0, op0=Alu.mult, op1=Alu.add)

            # ---- refractory: r[j,u] = sum_{w=1..4} z[j,u+w] (z>=0, padded)
            fz = feat[:, T + 1:T + 1 + T]
            cstride = fz.ap[1][0]
            win = bass.AP(tensor=fz.tensor, offset=fz.offset,
                          ap=[fz.ap[0], [cstride, T], [cstride, N_REF - 1]])
            r = work.tile([NH, T], F32, tag="r")
            nc.vector.tensor_reduce(r[:], win, mybir.AxisListType.X, Alu.add)

            # G = forward scan (in reversed time) of L: g = 0.8*g + L[u]
            G = work.tile([NH, T], F32, tag="G")
            nc.vector.tensor_tensor_scan(
                out=G[:], data0=dk[:, 0:T], data1=L_ps[:], initial=0.0,
                op0=Alu.mult, op1=Alu.add)

            # ---- psi (without the 0.5 factor; host rescales) ----
            psi_a = work.tile([NH, T], F32, tag="psi_a")
            nc.scalar.activation(psi_a[:], vT, Act.Abs,
                                 bias=bm1[:], scale=1.0 / THRESHOLD)
            psi = work.tile([NH, T], F32, tag="psi")
            nc.scalar.activation(psi[:], psi_a[:], Act.Relu,
                                 bias=1.0, scale=-1.0)

            # post = (r == 0) * psi;  M = post * (G + c2)
            post = work.tile([NH, T], F32, tag="post")
            nc.vector.scalar_tensor_tensor(
                out=post[:], in0=r[:], scalar=0.0, in1=psi[:],
                op0=Alu.is_equal, op1=Alu.mult)
            M = work.tile([NH, T], F32, tag="M")
            nc.vector.scalar_tensor_tensor(
                out=M[:], in0=G[:], scalar=c2, in1=post[:],
                op0=Alu.add, op1=Alu.mult)
            # Mf = scan(M), chunked+chained so transposes start early
            Mf = work.tile([NH, T], F32, tag="Mf")
            for q in range(Q):
                nc.vector.tensor_tensor_scan(
                    out=Mf[:, q * TC:(q + 1) * TC], data0=dk[:, 0:TC],
                    data1=M[:, q * TC:(q + 1) * TC],
                    initial=0.0 if q == 0 else Mf[:, q * TC - 1:q * TC],
                    op0=Alu.mult, op1=Alu.add)

            # ---- stationary operand rhsP = [e1fS | MfT] per chunk ----
            rhsP = work.tile([TC, Q, RP_W], F32, tag="rhsP")
            dwT_ps = psAcc.tile([RP_W, LHS_W], F32, tag="dwT")
            for q in range(Q):
                trE = psA.tile([TC, K], F32, tag="psA")
                nc.tensor.transpose(
                    trE[:], e1fp[:, q * TC + 1:(q + 1) * TC + 1],
                    ident[:K, :K])
                nc.scalar.copy(rhsP[:, q, 0:K], trE[:])
            for q in range(Q):
                trT = psA.tile([TC, NH], F32, tag="psA")
                nc.tensor.transpose(trT[:], Mf[:, q * TC:(q + 1) * TC],
                                    ident[:])
                nc.scalar.copy(rhsP[:, q, K:RP_W], trT[:])
            for q in range(Q):
                nc.tensor.matmul(dwT_ps[:], rhsP[:, q, :], lhs[:, q, :],
                                 start=(q == 0), stop=(q == Q - 1))

            # ---- pack transposed gradients and store ----
            # (dw_outT sits inside rows 0:K, cols NIN:NIN+NH of dwT)
            outt = work.tile([OUT_P, OUT_W], F32, tag="outt")
            nc.vector.tensor_copy(outt[0:64, :], dwT_ps[0:64, :])
            nc.scalar.copy(outt[64:OUT_P, :], dwT_ps[64:RP_W, :])
            nc.sync.dma_start(out=out_d.ap(), in_=outt[:])

    nc.compile()
    return nc


_NC_CACHE = None


def _get_nc():
    global _NC_CACHE
    if _NC_CACHE is None:
        _NC_CACHE = _build_program()
    return _NC_CACHE


def _prep_core_inputs(v, z, x, error1, error2, w_out, b, h):
    jsl = slice(h * NH, (h + 1) * NH)
    osl = slice((1 - h) * NH, (2 - h) * NH)
    rev = slice(None, None, -1)

    z_r = z[b, rev, :]                         # [T, N]
    x_r = x[b, rev, :]                         # [T, NIN]
    zsh = np.vstack([z_r[1:], np.zeros((1, N), np.float32)])

    feat = np.zeros((NH, FEAT_W), np.float32)
    feat[:, 0:T] = v[b, rev, jsl].T
    feat[:, T:2 * T] = z_r[:, jsl].T           # cols 600:604 stay zero
    feat[:, FEAT_W - 1] = (REG / (B * T)) * error2[jsl]

    lhs = np.empty((TC, Q, LHS_W), np.float32)
    for q in range(Q):
        rows = slice(q * TC, (q + 1) * TC)
        lhs[:, q, 0:NIN] = x_r[rows]
        lhs[:, q, NIN:NIN + NH] = zsh[rows][:, jsl]
        lhs[:, q, NIN + NH:LHS_W] = zsh[rows][:, osl]

    small = np.zeros((K, SMALL_W), np.float32)
    small[:, 0:T] = error1[b, rev, :].T        # col T stays zero (scan pad)
    small[:, T + 1:T + 1 + NH] = w_out[jsl, :].T

    # rank-1 dw_out correction (pure-input term dropped by the zsh shift):
    # dw_out[jsl] += z_u[0] (x) e1f[0] = z[b,T-1,jsl] (x) error1[b,T-1,:]
    corr = np.outer(z_r[0, jsl], error1[b, T - 1, :]).astype(np.float32)

    return {"feat": feat, "lhsT": np.ascontiguousarray(lhs),
            "small": small}, corr


def kernel(v, z, x, error1, error2, w_out, _trace=False):
    v = np.asarray(v, np.float32)
    z = np.asarray(z, np.float32)
    x = np.asarray(x, np.float32)
    error1 = np.asarray(error1, np.float32)
    error2 = np.asarray(error2, np.float32)
    w_out = np.asarray(w_out, np.float32)

    nc = _get_nc()
    prepped = [_prep_core_inputs(v, z, x, error1, error2, w_out, c // 2, c % 2)
               for c in range(N_CORES)]
    in_maps = [p[0] for p in prepped]
    corrs = [p[1] for p in prepped]
    res = run_bass_kernel_spmd(nc, in_maps, core_ids=list(range(N_CORES)),
                               trace=_trace)

    dw_in = np.zeros((NIN, N), np.float32)
    dw_rec = np.zeros((N, N), np.float32)
    dw_out = np.zeros((N, K), np.float32)
    for h in range(2):
        jsl = slice(h * NH, (h + 1) * NH)
        osl = slice((1 - h) * NH, (2 - h) * NH)
        s = np.zeros((OUT_P, OUT_W), np.float64)
        for b in range(B):
            s += res.results[2 * b + h]["out"]
        dw_in[:, jsl] = 0.5 * s[K:RP_W, 0:NIN].T
        dw_rec[jsl, jsl] = 0.5 * s[K:RP_W, NIN:NIN + NH].T    # own-half rows
        dw_rec[osl, jsl] = 0.5 * s[K:RP_W, NIN + NH:LHS_W].T  # other-half
        dw_out[jsl, :] = s[0:K, NIN:NIN + NH].T
        for b in range(B):
            dw_out[jsl, :] += corrs[2 * b + h]
    np.fill_diagonal(dw_rec, 0.0)

    if _trace:
        return (dw_in, dw_rec, dw_out), res
    return dw_in, dw_rec, dw_out
